# revision 37
# baseline (speedup 1.0000x reference)
"""Multi-head attention (shared key head) on 8 TRN2 NeuronCores.

Sharding: core c handles batch b = c % 4 and head group g = c // 4
(heads 4g..4g+3).  Per-core weights are sliced on host; x is
pre-transposed (and bf16-cast) on host so the device never transposes
the activations.

Device-side per core (bf16 matmul path, fp32 PSUM accumulation):
  xT [512, 2048] -> QT [a, s] (2 tiles, 2 heads each), KT zero-padded
  into two [128, s] variants (low/high partition half) so every scores
  matmul contracts over the full 128 partitions -- half-array matmuls
  keep the PE activity monitor from ever unthrottling the clock.
  V per s-tile [128, 4*128]: per-head 128-col block = [V+bv | ones | 0].
  scores^T[k, q] = KTz^T . QT  (k on partitions, q moving, 1024-chunks)
  attn^T = exp(scale * scores^T)  (no max subtraction: |scores| < ~0.3)
  causal: k-tiles beyond the chunk's causal extent skipped, diagonal
  tiles restrict the moving range, one triangular mask multiply on the
  128-wide boundary block.
  out^T[o(+denom), q] accumulates Vaug^T . attn^T in PSUM; ones column
  of Vaug yields the softmax denominator in row 64.
  Epilogue (no PE, no extra PSUM): denom row -> partition-major via DMA
  reshape, reciprocal, back to row-major, GPSIMD partition-broadcast,
  one DVE multiply; output stays [o, q] and the host transposes it
  during the unshard.
"""

import math
import numpy as np
import ml_dtypes

import concourse.bass as bass
import concourse.mybir as mybir
import concourse.tile as tile
from concourse import bacc
from concourse.bass_utils import run_bass_kernel_spmd

B, S, D = 4, 2048, 512
H, A, O = 8, 64, 64
NCORES = 8
HPC = 4                # heads per core
APC = HPC * A          # 256 projection cols per core
VBLK = 128             # per-head V block width (64 out + 1 ones + 63 zero)
SCALE = 1.0 / math.sqrt(S)

F32 = mybir.dt.float32
BF16 = mybir.dt.bfloat16
AF = mybir.ActivationFunctionType
BF_NP = ml_dtypes.bfloat16

QC = 1024              # attention q-chunk width
N_QC = S // QC         # 2
N_DT = D // 128        # 4 contraction tiles
N_SC = S // 512        # 4 s-chunks of 512
N_ST = S // 128        # 16 s-tiles / k-tiles of 128


def build():
    nc = bacc.Bacc("TRN2", target_bir_lowering=False, debug=False,
                   num_devices=NCORES)

    xT_d = nc.dram_tensor("xT", [D, S], BF16, kind="ExternalInput").ap()
    wq_d = nc.dram_tensor("wq", [D, APC], BF16, kind="ExternalInput").ap()
    bq_d = nc.dram_tensor("bq", [2, 128, 1], F32, kind="ExternalInput").ap()
    wk_d = nc.dram_tensor("wk", [D, A], BF16, kind="ExternalInput").ap()
    wv_d = nc.dram_tensor("wv", [D, APC], BF16, kind="ExternalInput").ap()
    bvm_d = nc.dram_tensor("bvm", [128, HPC * VBLK], BF16,
                           kind="ExternalInput").ap()
    out_d = nc.dram_tensor("out", [HPC, N_QC, O, QC], F32,
                       kind="ExternalOutput").ap()

    ngI_d = nc.inline_tensor((np.eye(128) * -1e9).astype(BF_NP), "ngI").ap()
    mlt_np = (np.arange(128)[None, :] < np.arange(128)[:, None])
    mlt_d = nc.inline_tensor(mlt_np.astype(BF_NP), "mlt").ap()

    with tile.TileContext(nc) as tc:
        with tc.tile_pool(name="const", bufs=1) as cpool, \
             tc.tile_pool(name="persist", bufs=1) as ppool, \
             tc.tile_pool(name="attn", bufs=34) as apool, \
             tc.tile_pool(name="fin", bufs=4) as fpool, \
             tc.tile_pool(name="ps_sc", bufs=2, space="PSUM") as ps_sc, \
             tc.tile_pool(name="ps_av", bufs=2, space="PSUM") as ps_av:

            # ---- constants / weights to SBUF ----
            ngI = cpool.tile([128, 128], BF16, tag="ngI", name="ngI")
            mlt = cpool.tile([128, 128], BF16, tag="mlt", name="mlt")
            bvm = cpool.tile([128, HPC * VBLK], BF16, tag="bvm", name="bvm")

            wq_sb, wk_sb, wv_sb = [], [], []
            for dt in range(N_DT):
                wq_sb.append(cpool.tile([128, APC], BF16, tag=f"wq{dt}", name=f"wq{dt}"))
                wk_sb.append(cpool.tile([128, A], BF16, tag=f"wk{dt}", name=f"wk{dt}"))
                wv_sb.append(cpool.tile([128, APC], BF16, tag=f"wv{dt}", name=f"wv{dt}"))
            bq_sb = [cpool.tile([128, 1], F32, tag=f"bq{at}", name=f"bq{at}")
                     for at in range(2)]

            # ---- x^T half tiles (declared here; DMA order below) ----
            xth = [[ppool.tile([128, QC], BF16, tag=f"xt{dt}_{sp}",
                               name=f"xt{dt}_{sp}") for sp in range(2)]
                   for dt in range(N_DT)]

            # DMA order: everything the first projections need comes first,
            # round-robin across the three queues
            SY, SC, GP = nc.sync, nc.scalar, nc.gpsimd
            order = []
            for dt in range(N_DT):        # x^T first half
                order.append((xth[dt][0][:, :],
                              xT_d[dt * 128:(dt + 1) * 128, 0:QC]))
            for dt in range(N_DT):
                order.append((wq_sb[dt][:, :],
                              wq_d[dt * 128:(dt + 1) * 128, :]))
            for at in range(2):
                order.append((bq_sb[at][:, :], bq_d[at]))
            for dt in range(N_DT):
                order.append((wk_sb[dt][:, :],
                              wk_d[dt * 128:(dt + 1) * 128, :]))
            for dt in range(N_DT):        # x^T second half
                order.append((xth[dt][1][:, :],
                              xT_d[dt * 128:(dt + 1) * 128, QC:S]))
            for dt in range(N_DT):
                order.append((wv_sb[dt][:, :],
                              wv_d[dt * 128:(dt + 1) * 128, :]))
            order += [(ngI[:, :], ngI_d[:, :]), (mlt[:, :], mlt_d[:, :]),
                      (bvm[:, :], bvm_d[:, :])]
            for i, (dst, srcap) in enumerate(order):
                [SY, SC, GP][i % 3].dma_start(out=dst, in_=srcap)

            # PE warm-up: full-array dummy matmuls on the first weight tile
            # keep the activity monitor unthrottled while x^T DMAs land
            wu = ps_sc.tile([128, APC], F32, tag="sc", name="wu")
            for i in range(10):
                nc.tensor.matmul(out=wu[:, :], lhsT=wq_sb[0][:, 0:128],
                                 rhs=wq_sb[0][:, :], start=True, stop=True)

            # ---- projections ----
            # QT: [a, s] packed 2 heads per 128-partition tile, half tiles
            qt = [[ppool.tile([128, QC], BF16, tag=f"qt{at}_{sp}",
                              name=f"qt{at}_{sp}") for sp in range(2)]
                  for at in range(2)]
            def qt_proj(at, sp):
                ps = ps_av.tile([128, 1024], F32, tag="av", name="qps")
                for hh in range(2):
                    hs = slice(hh * 512, (hh + 1) * 512)
                    for dt in range(N_DT):
                        nc.tensor.matmul(
                            out=ps[:, hs],
                            lhsT=wq_sb[dt][:, at * 128:(at + 1) * 128],
                            rhs=xth[dt][sp][:, hs],
                            start=(dt == 0), stop=(dt == N_DT - 1))
                nc.vector.tensor_scalar_add(out=qt[at][sp][:, :],
                                            in0=ps[:, :],
                                            scalar1=bq_sb[at][:, :])

            # KT zero-padded into both partition halves: ktz[0] has K^T in
            # rows 0..63 (even heads), ktz[1] in rows 64..127 (odd heads).
            # Full-128 contraction keeps the PE activity monitor warm.
            ktz = [[ppool.tile([128, QC], BF16, tag=f"ktz{i}_{sp}",
                               name=f"ktz{i}_{sp}") for sp in range(2)]
                   for i in range(2)]
            for sp in range(2):
                nc.gpsimd.memset(ktz[0][sp][64:128, :], 0.0)
                nc.gpsimd.memset(ktz[1][sp][0:64, :], 0.0)
            def kt_proj(sp):
                ps = ps_av.tile([64, 1024], F32, tag="av", name="kps")
                for hh in range(2):
                    hs = slice(hh * 512, (hh + 1) * 512)
                    for dt in range(N_DT):
                        nc.tensor.matmul(out=ps[:, hs], lhsT=wk_sb[dt][:, :],
                                         rhs=xth[dt][sp][:, hs],
                                         start=(dt == 0), stop=(dt == N_DT - 1))
                nc.vector.tensor_copy(ktz[0][sp][0:64, :], ps[:, :])
                nc.vector.tensor_copy(ktz[1][sp][64:128, :], ps[:, :])

            # V: per s-tile [128, 4*128]; block = [V+bv | ones | zeros]
            vt = []
            for st in range(N_ST):
                t = ppool.tile([128, HPC * VBLK], BF16, tag=f"v{st}",
                               name=f"v{st}")
                v3 = t[:, :].rearrange("p (h c) -> p h c", h=HPC)
                b3 = bvm[:, :].rearrange("p (h c) -> p h c", h=HPC)
                nc.vector.tensor_copy(v3[:, :, O:VBLK], b3[:, :, O:VBLK])
                vt.append(t)
            def v_proj(st):
                ps = ps_av.tile([128, APC], F32, tag="av", name="vps")
                sp, so = st // 8, (st % 8) * 128
                for dt in range(N_DT):
                    nc.tensor.matmul(
                        out=ps[:, :],
                        lhsT=xth[dt][sp][:, so:so + 128],
                        rhs=wv_sb[dt][:, :],
                        start=(dt == 0), stop=(dt == N_DT - 1))
                for h in range(HPC):
                    nc.vector.tensor_add(
                        out=vt[st][:, h * VBLK:h * VBLK + O],
                        in0=ps[:, h * O:(h + 1) * O],
                        in1=bvm[:, h * VBLK:h * VBLK + O])

            # ---- attention ----
            def sc_exp(h, qc):
                """scores + exp for one (head, q-chunk); returns atn tiles"""
                at = h // 2
                nkj = (QC // 128) * (qc + 1)
                atns = []
                for kj in range(nkj):
                    m = kj - (QC // 128) * qc
                    vs = 128 * m if m > 0 else 0     # valid q start
                    qlo = qc * QC
                    sc_ps = ps_sc.tile([128, QC], F32, tag="sc", name="sc")
                    for hf in range(QC // 512):
                        lo = max(vs, hf * 512)
                        hi = (hf + 1) * 512
                        if lo >= hi:
                            continue
                        nc.tensor.matmul(
                            out=sc_ps[:, lo:hi],
                            lhsT=ktz[h % 2][kj // 8][:, (kj % 8) * 128:
                                                     (kj % 8 + 1) * 128],
                            rhs=qt[at][qc][:, lo:hi],
                            start=True, stop=True)
                    if m >= 0:
                        nc.tensor.matmul(out=sc_ps[:, vs:vs + 128],
                                         lhsT=ngI[:, :], rhs=mlt[:, :],
                                         start=False, stop=True,
                                         skip_group_check=True)
                    atn = apool.tile([128, QC], BF16, tag="atn", name="atn")
                    nc.scalar.activation(out=atn[:, vs:QC],
                                         in_=sc_ps[:, vs:QC],
                                         func=AF.Exp, scale=SCALE)
                    atns.append(atn)
                return atns

            def av_part(h, qc, atns):
                """V-weighted accumulation + epilogue for one chunk"""
                av = ps_av.tile([128, QC], F32, tag="av", name="av")
                nkj = (QC // 128) * (qc + 1)
                for kj in range(nkj):
                    m = kj - (QC // 128) * qc
                    vs = 128 * m if m > 0 else 0
                    for hf in range(QC // 512):
                        lo = max(vs, hf * 512)
                        hi = (hf + 1) * 512
                        if lo >= hi:
                            continue
                        last_kj = nkj - 1 if hf == 1 else \
                            (QC // 128) * qc + 3
                        nc.tensor.matmul(
                            out=av[:, lo:hi],
                            lhsT=vt[kj][:, h * VBLK:(h + 1) * VBLK],
                            rhs=atns[kj][:, lo:hi],
                            start=(kj == 0), stop=(kj == last_kj))

                # per-bank-half epilogue: the low half's accumulation group
                # closes several k-tiles before the high half's, so its
                # normalize/DMA overlaps the remaining AV matmuls
                for hf in range(QC // 512):
                    hs = slice(hf * 512, (hf + 1) * 512)
                    dr = fpool.tile([1, 512], F32, tag="dr", name="dr")
                    nc.vector.tensor_copy(dr[:, :], av[O:O + 1, hs])
                    drr = fpool.tile([1, 512], F32, tag="drr", name="drr")
                    nc.vector.reciprocal_approx_fast(out=drr[:, :],
                                                     in_=dr[:, :])
                    rb = fpool.tile([O, 512], F32, tag="rb", name="rb")
                    nc.gpsimd.partition_broadcast(rb[:, :], drr[:, :],
                                                  channels=O)
                    ov = fpool.tile([O, 512], F32, tag="ov", name="ov")
                    nc.vector.tensor_mul(ov[:, :], av[0:O, hs], rb[:, :])
                    nc.scalar.dma_start(out=out_d[h, qc, :, hs], in_=ov[:, :])

            def attn_chunk(h, qc):
                av_part(h, qc, sc_exp(h, qc))

            # schedule: minimal projection -> early scores/exp, then a
            # chunk-level software pipeline: scores/exp of chunk i+1 are
            # emitted before the AV+epilogue of chunk i so the scalar
            # engine's exp stream never stalls at chunk boundaries
            qt_proj(0, 0)
            kt_proj(0)
            a00 = sc_exp(0, 0)
            a10 = sc_exp(1, 0)
            qt_proj(0, 1)
            qt_proj(1, 0)
            qt_proj(1, 1)
            kt_proj(1)
            a20 = sc_exp(2, 0)
            for st in range(8):
                v_proj(st)
            av_part(0, 0, a00)
            a01 = sc_exp(0, 1)
            for st in range(8, 12):
                v_proj(st)
            av_part(1, 0, a10)
            a11 = sc_exp(1, 1)
            for st in range(12, N_ST):
                v_proj(st)
            av_part(2, 0, a20)
            a21 = sc_exp(2, 1)
            av_part(0, 1, a01)
            a31 = sc_exp(3, 1)
            av_part(1, 1, a11)
            a30 = sc_exp(3, 0)
            av_part(2, 1, a21)
            av_part(3, 1, a31)
            av_part(3, 0, a30)

    nc.compile()
    return nc


_NC = None
LAST_RESULTS = None


def _bvm(bv_slice):
    blk = np.zeros((HPC, VBLK), dtype=np.float32)
    blk[:, :O] = np.asarray(bv_slice, dtype=np.float32).reshape(HPC, O)
    blk[:, O] = 1.0
    return np.ascontiguousarray(np.broadcast_to(
        blk.reshape(1, HPC * VBLK), (128, HPC * VBLK))).astype(BF_NP)


def make_in_maps(x, Wq, bq, Wk, Wv, bv):
    in_maps = []
    for c in range(NCORES):
        b, g = c % 4, c // 4
        cols = slice(g * APC, (g + 1) * APC)
        in_maps.append({
            "xT": np.ascontiguousarray(x[b].T).astype(BF_NP),
            "wq": np.ascontiguousarray(Wq[:, cols]).astype(BF_NP),
            "bq": np.ascontiguousarray(bq[cols].reshape(2, 128, 1)),
            "wk": np.ascontiguousarray(Wk).astype(BF_NP),
            "wv": np.ascontiguousarray(Wv[:, cols]).astype(BF_NP),
            "bvm": _bvm(bv[cols]),
        })
    return in_maps


def gather_out(results):
    out = np.empty((B, S, H * O), dtype=np.float32)
    for c in range(NCORES):
        b, g = c % 4, c // 4
        oc = results[c]["out"]          # [HPC, N_QC, O, QC]
        for h in range(HPC):
            col = g * APC + h * O
            for qc in range(N_QC):
                out[b, qc * QC:(qc + 1) * QC, col:col + O] = oc[h, qc].T
    return out


def kernel(**inputs):
    global _NC, LAST_RESULTS
    x = np.asarray(inputs["x"], dtype=np.float32)
    Wq = np.asarray(inputs["Wq"], dtype=np.float32)
    bq = np.asarray(inputs["bq"], dtype=np.float32)
    Wk = np.asarray(inputs["Wk"], dtype=np.float32)
    Wv = np.asarray(inputs["Wv"], dtype=np.float32)
    bv = np.asarray(inputs["bv"], dtype=np.float32)

    if _NC is None:
        _NC = build()

    in_maps = make_in_maps(x, Wq, bq, Wk, Wv, bv)
    res = run_bass_kernel_spmd(_NC, in_maps, core_ids=list(range(NCORES)))
    LAST_RESULTS = res
    return gather_out(res.results)


# revision 38
# speedup vs baseline: 1.1580x; 1.1580x over previous
"""Multi-head attention (shared key head) on 8 TRN2 NeuronCores.

Sharding: core c handles batch b = c % 4 and head group g = c // 4
(heads 4g..4g+3).  Per-core weights are sliced on host; x is
pre-transposed (and bf16-cast) on host so the device never transposes
the activations.

Device-side per core (bf16 matmul path, fp32 PSUM accumulation):
  xT [512, 2048] -> QT [a, s] (2 tiles, 2 heads each), KT zero-padded
  into two [128, s] variants (low/high partition half) so every scores
  matmul contracts over the full 128 partitions -- half-array matmuls
  keep the PE activity monitor from ever unthrottling the clock.
  V per s-tile [128, 4*128]: per-head 128-col block = [V+bv | ones | 0].
  scores^T[k, q] = KTz^T . QT  (k on partitions, q moving, 1024-chunks)
  attn^T = exp(scale * scores^T)  (no max subtraction: |scores| < ~0.3)
  causal: k-tiles beyond the chunk's causal extent skipped, diagonal
  tiles restrict the moving range, one triangular mask multiply on the
  128-wide boundary block.
  out^T[o(+denom), q] accumulates Vaug^T . attn^T in PSUM; ones column
  of Vaug yields the softmax denominator in row 64.
  Epilogue (no PE, no extra PSUM): denom row -> partition-major via DMA
  reshape, reciprocal, back to row-major, GPSIMD partition-broadcast,
  one DVE multiply; output stays [o, q] and the host transposes it
  during the unshard.
"""

import math
import numpy as np
import ml_dtypes

import concourse.bass as bass
import concourse.mybir as mybir
import concourse.tile as tile
from concourse import bacc
from concourse.bass_utils import run_bass_kernel_spmd

B, S, D = 4, 2048, 512
H, A, O = 8, 64, 64
NCORES = 8
HPC = 4                # heads per core
APC = HPC * A          # 256 projection cols per core
VBLK = 128             # per-head V block width (64 out + 1 ones + 63 zero)
SCALE = 1.0 / math.sqrt(S)

F32 = mybir.dt.float32
BF16 = mybir.dt.bfloat16
AF = mybir.ActivationFunctionType
BF_NP = ml_dtypes.bfloat16

QC = 1024              # attention q-chunk width
N_QC = S // QC         # 2
N_DT = D // 128        # 4 contraction tiles
N_SC = S // 512        # 4 s-chunks of 512
N_ST = S // 128        # 16 s-tiles / k-tiles of 128


def build():
    nc = bacc.Bacc("TRN2", target_bir_lowering=False, debug=False,
                   num_devices=NCORES)

    xT_d = nc.dram_tensor("xT", [D, S], BF16, kind="ExternalInput").ap()
    wq_d = nc.dram_tensor("wq", [D, APC], BF16, kind="ExternalInput").ap()
    bq_d = nc.dram_tensor("bq", [2, 128, 1], F32, kind="ExternalInput").ap()
    wk_d = nc.dram_tensor("wk", [D, A], BF16, kind="ExternalInput").ap()
    wv_d = nc.dram_tensor("wv", [D, APC], BF16, kind="ExternalInput").ap()
    bvm_d = nc.dram_tensor("bvm", [128, HPC * VBLK], BF16,
                           kind="ExternalInput").ap()
    out_d = nc.dram_tensor("out", [HPC, N_QC, O, QC], F32,
                       kind="ExternalOutput").ap()

    ngI_d = nc.inline_tensor((np.eye(128) * -1e9).astype(BF_NP), "ngI").ap()
    mlt_np = (np.arange(128)[None, :] < np.arange(128)[:, None])
    mlt_d = nc.inline_tensor(mlt_np.astype(BF_NP), "mlt").ap()

    with tile.TileContext(nc) as tc:
        with tc.tile_pool(name="const", bufs=1) as cpool, \
             tc.tile_pool(name="persist", bufs=1) as ppool, \
             tc.tile_pool(name="attn", bufs=34) as apool, \
             tc.tile_pool(name="fin", bufs=4) as fpool, \
             tc.tile_pool(name="ps_sc", bufs=2, space="PSUM") as ps_sc, \
             tc.tile_pool(name="ps_av", bufs=2, space="PSUM") as ps_av:

            # ---- constants / weights to SBUF ----
            ngI = cpool.tile([128, 128], BF16, tag="ngI", name="ngI")
            mlt = cpool.tile([128, 128], BF16, tag="mlt", name="mlt")
            bvm = cpool.tile([128, HPC * VBLK], BF16, tag="bvm", name="bvm")

            wq_sb, wk_sb, wv_sb = [], [], []
            for dt in range(N_DT):
                wq_sb.append(cpool.tile([128, APC], BF16, tag=f"wq{dt}", name=f"wq{dt}"))
                wk_sb.append(cpool.tile([128, A], BF16, tag=f"wk{dt}", name=f"wk{dt}"))
                wv_sb.append(cpool.tile([128, APC], BF16, tag=f"wv{dt}", name=f"wv{dt}"))
            bq_sb = [cpool.tile([128, 1], F32, tag=f"bq{at}", name=f"bq{at}")
                     for at in range(2)]

            # ---- x^T half tiles (declared here; DMA order below) ----
            xth = [[ppool.tile([128, QC], BF16, tag=f"xt{dt}_{sp}",
                               name=f"xt{dt}_{sp}") for sp in range(2)]
                   for dt in range(N_DT)]

            # DMA order: everything the first projections need comes first,
            # round-robin across the three queues
            SY, SC, GP = nc.sync, nc.scalar, nc.gpsimd
            order = []
            for dt in range(N_DT):        # x^T first half
                order.append((xth[dt][0][:, :],
                              xT_d[dt * 128:(dt + 1) * 128, 0:QC]))
            for dt in range(N_DT):
                order.append((wq_sb[dt][:, :],
                              wq_d[dt * 128:(dt + 1) * 128, :]))
            for at in range(2):
                order.append((bq_sb[at][:, :], bq_d[at]))
            for dt in range(N_DT):
                order.append((wk_sb[dt][:, :],
                              wk_d[dt * 128:(dt + 1) * 128, :]))
            for dt in range(N_DT):        # x^T second half
                order.append((xth[dt][1][:, :],
                              xT_d[dt * 128:(dt + 1) * 128, QC:S]))
            for dt in range(N_DT):
                order.append((wv_sb[dt][:, :],
                              wv_d[dt * 128:(dt + 1) * 128, :]))
            order += [(ngI[:, :], ngI_d[:, :]), (mlt[:, :], mlt_d[:, :]),
                      (bvm[:, :], bvm_d[:, :])]
            for i, (dst, srcap) in enumerate(order):
                [SY, SC, GP][i % 3].dma_start(out=dst, in_=srcap)

            # PE warm-up: full-array dummy matmuls on the first weight tile
            # keep the activity monitor unthrottled while x^T DMAs land
            wu = ps_sc.tile([128, APC], F32, tag="sc", name="wu")
            for i in range(10):
                nc.tensor.matmul(out=wu[:, :], lhsT=wq_sb[0][:, 0:128],
                                 rhs=wq_sb[0][:, :], start=True, stop=True)

            # ---- projections ----
            # QT: [a, s] packed 2 heads per 128-partition tile, half tiles
            qt = [[ppool.tile([128, QC], BF16, tag=f"qt{at}_{sp}",
                              name=f"qt{at}_{sp}") for sp in range(2)]
                  for at in range(2)]
            def qt_proj(at, sp):
                ps = ps_av.tile([128, 1024], F32, tag="av", name="qps")
                for hh in range(2):
                    hs = slice(hh * 512, (hh + 1) * 512)
                    for dt in range(N_DT):
                        nc.tensor.matmul(
                            out=ps[:, hs],
                            lhsT=wq_sb[dt][:, at * 128:(at + 1) * 128],
                            rhs=xth[dt][sp][:, hs],
                            start=(dt == 0), stop=(dt == N_DT - 1))
                nc.vector.tensor_scalar_add(out=qt[at][sp][:, :],
                                            in0=ps[:, :],
                                            scalar1=bq_sb[at][:, :])

            # KT zero-padded into both partition halves: ktz[0] has K^T in
            # rows 0..63 (even heads), ktz[1] in rows 64..127 (odd heads).
            # Full-128 contraction keeps the PE activity monitor warm.
            ktz = [[ppool.tile([128, QC], BF16, tag=f"ktz{i}_{sp}",
                               name=f"ktz{i}_{sp}") for sp in range(2)]
                   for i in range(2)]
            for sp in range(2):
                nc.gpsimd.memset(ktz[0][sp][64:128, :], 0.0)
                nc.gpsimd.memset(ktz[1][sp][0:64, :], 0.0)
            def kt_proj(sp):
                ps = ps_av.tile([64, 1024], F32, tag="av", name="kps")
                for hh in range(2):
                    hs = slice(hh * 512, (hh + 1) * 512)
                    for dt in range(N_DT):
                        nc.tensor.matmul(out=ps[:, hs], lhsT=wk_sb[dt][:, :],
                                         rhs=xth[dt][sp][:, hs],
                                         start=(dt == 0), stop=(dt == N_DT - 1))
                nc.vector.tensor_copy(ktz[0][sp][0:64, :], ps[:, :])
                nc.vector.tensor_copy(ktz[1][sp][64:128, :], ps[:, :])

            # V: per s-tile [128, 4*128]; block = [V+bv | ones | zeros]
            vt = []
            for st in range(N_ST):
                t = ppool.tile([128, HPC * VBLK], BF16, tag=f"v{st}",
                               name=f"v{st}")
                v3 = t[:, :].rearrange("p (h c) -> p h c", h=HPC)
                b3 = bvm[:, :].rearrange("p (h c) -> p h c", h=HPC)
                nc.vector.tensor_copy(v3[:, :, O:VBLK], b3[:, :, O:VBLK])
                vt.append(t)
            def v_proj(st):
                ps = ps_av.tile([128, APC], F32, tag="av", name="vps")
                sp, so = st // 8, (st % 8) * 128
                for dt in range(N_DT):
                    nc.tensor.matmul(
                        out=ps[:, :],
                        lhsT=xth[dt][sp][:, so:so + 128],
                        rhs=wv_sb[dt][:, :],
                        start=(dt == 0), stop=(dt == N_DT - 1))
                for h in range(HPC):
                    nc.vector.tensor_add(
                        out=vt[st][:, h * VBLK:h * VBLK + O],
                        in0=ps[:, h * O:(h + 1) * O],
                        in1=bvm[:, h * VBLK:h * VBLK + O])

            # ---- attention ----
            def sc_exp(h, qc):
                """scores + exp for one (head, q-chunk); returns atn tiles"""
                at = h // 2
                nkj = (QC // 128) * (qc + 1)
                atns = []
                for kj in range(nkj):
                    m = kj - (QC // 128) * qc
                    vs = 128 * m if m > 0 else 0     # valid q start
                    qlo = qc * QC
                    sc_ps = ps_sc.tile([128, QC], F32, tag="sc", name="sc")
                    for hf in range(QC // 512):
                        lo = max(vs, hf * 512)
                        hi = (hf + 1) * 512
                        if lo >= hi:
                            continue
                        nc.tensor.matmul(
                            out=sc_ps[:, lo:hi],
                            lhsT=ktz[h % 2][kj // 8][:, (kj % 8) * 128:
                                                     (kj % 8 + 1) * 128],
                            rhs=qt[at][qc][:, lo:hi],
                            start=True, stop=True)
                    if m >= 0:
                        nc.tensor.matmul(out=sc_ps[:, vs:vs + 128],
                                         lhsT=ngI[:, :], rhs=mlt[:, :],
                                         start=False, stop=True,
                                         skip_group_check=True)
                    atn = apool.tile([128, QC], BF16, tag="atn", name="atn")
                    nc.scalar.activation(out=atn[:, vs:QC],
                                         in_=sc_ps[:, vs:QC],
                                         func=AF.Exp, scale=SCALE)
                    atns.append(atn)
                return atns

            def av_part(h, qc, atns):
                """V-weighted accumulation + epilogue for one chunk"""
                av = ps_av.tile([128, QC], F32, tag="av", name="av")
                nkj = (QC // 128) * (qc + 1)
                for kj in range(nkj):
                    m = kj - (QC // 128) * qc
                    vs = 128 * m if m > 0 else 0
                    for hf in range(QC // 512):
                        lo = max(vs, hf * 512)
                        hi = (hf + 1) * 512
                        if lo >= hi:
                            continue
                        last_kj = nkj - 1 if hf == 1 else \
                            (QC // 128) * qc + 3
                        nc.tensor.matmul(
                            out=av[:, lo:hi],
                            lhsT=vt[kj][:, h * VBLK:(h + 1) * VBLK],
                            rhs=atns[kj][:, lo:hi],
                            start=(kj == 0), stop=(kj == last_kj))

                dr = fpool.tile([1, QC], F32, tag="dr", name="dr")
                nc.vector.tensor_copy(dr[:, :], av[O:O + 1, :])
                drr = fpool.tile([1, QC], F32, tag="drr", name="drr")
                nc.vector.reciprocal_approx_fast(out=drr[:, :], in_=dr[:, :])
                rb = fpool.tile([O, QC], F32, tag="rb", name="rb")
                nc.gpsimd.partition_broadcast(rb[:, :], drr[:, :], channels=O)
                ov = fpool.tile([O, QC], F32, tag="ov", name="ov")
                nc.vector.tensor_mul(ov[:, :], av[0:O, :], rb[:, :])
                nc.scalar.dma_start(out=out_d[h, qc], in_=ov[:, :])

            def attn_chunk(h, qc):
                av_part(h, qc, sc_exp(h, qc))

            # schedule: minimal projection -> early scores/exp, then a
            # chunk-level software pipeline: scores/exp of chunk i+1 are
            # emitted before the AV+epilogue of chunk i so the scalar
            # engine's exp stream never stalls at chunk boundaries
            qt_proj(0, 0)
            kt_proj(0)
            a00 = sc_exp(0, 0)
            a10 = sc_exp(1, 0)
            qt_proj(0, 1)
            qt_proj(1, 0)
            qt_proj(1, 1)
            kt_proj(1)
            a20 = sc_exp(2, 0)
            for st in range(8):
                v_proj(st)
            av_part(0, 0, a00)
            a01 = sc_exp(0, 1)
            for st in range(8, 12):
                v_proj(st)
            av_part(1, 0, a10)
            a11 = sc_exp(1, 1)
            for st in range(12, N_ST):
                v_proj(st)
            av_part(2, 0, a20)
            a21 = sc_exp(2, 1)
            av_part(0, 1, a01)
            a31 = sc_exp(3, 1)
            av_part(1, 1, a11)
            a30 = sc_exp(3, 0)
            av_part(2, 1, a21)
            av_part(3, 1, a31)
            av_part(3, 0, a30)

    nc.compile()
    return nc


_NC = None
LAST_RESULTS = None


def _bvm(bv_slice):
    blk = np.zeros((HPC, VBLK), dtype=np.float32)
    blk[:, :O] = np.asarray(bv_slice, dtype=np.float32).reshape(HPC, O)
    blk[:, O] = 1.0
    return np.ascontiguousarray(np.broadcast_to(
        blk.reshape(1, HPC * VBLK), (128, HPC * VBLK))).astype(BF_NP)


def make_in_maps(x, Wq, bq, Wk, Wv, bv):
    in_maps = []
    for c in range(NCORES):
        b, g = c % 4, c // 4
        cols = slice(g * APC, (g + 1) * APC)
        in_maps.append({
            "xT": np.ascontiguousarray(x[b].T).astype(BF_NP),
            "wq": np.ascontiguousarray(Wq[:, cols]).astype(BF_NP),
            "bq": np.ascontiguousarray(bq[cols].reshape(2, 128, 1)),
            "wk": np.ascontiguousarray(Wk).astype(BF_NP),
            "wv": np.ascontiguousarray(Wv[:, cols]).astype(BF_NP),
            "bvm": _bvm(bv[cols]),
        })
    return in_maps


def gather_out(results):
    out = np.empty((B, S, H * O), dtype=np.float32)
    for c in range(NCORES):
        b, g = c % 4, c // 4
        oc = results[c]["out"]          # [HPC, N_QC, O, QC]
        for h in range(HPC):
            col = g * APC + h * O
            for qc in range(N_QC):
                out[b, qc * QC:(qc + 1) * QC, col:col + O] = oc[h, qc].T
    return out


def kernel(**inputs):
    global _NC, LAST_RESULTS
    x = np.asarray(inputs["x"], dtype=np.float32)
    Wq = np.asarray(inputs["Wq"], dtype=np.float32)
    bq = np.asarray(inputs["bq"], dtype=np.float32)
    Wk = np.asarray(inputs["Wk"], dtype=np.float32)
    Wv = np.asarray(inputs["Wv"], dtype=np.float32)
    bv = np.asarray(inputs["bv"], dtype=np.float32)

    if _NC is None:
        _NC = build()

    in_maps = make_in_maps(x, Wq, bq, Wk, Wv, bv)
    res = run_bass_kernel_spmd(_NC, in_maps, core_ids=list(range(NCORES)))
    LAST_RESULTS = res
    return gather_out(res.results)


# revision 39
# speedup vs baseline: 1.2318x; 1.0637x over previous
"""Multi-head attention (shared key head) on 8 TRN2 NeuronCores.

Sharding: core c handles batch b = c % 4 and head group g = c // 4
(heads 4g..4g+3).  Per-core weights are sliced on host; x is
pre-transposed (and bf16-cast) on host so the device never transposes
the activations.

Device-side per core (bf16 matmul path, fp32 PSUM accumulation):
  xT [512, 2048] -> QT [a, s] (2 tiles, 2 heads each), KT zero-padded
  into two [128, s] variants (low/high partition half) so every scores
  matmul contracts over the full 128 partitions -- half-array matmuls
  keep the PE activity monitor from ever unthrottling the clock.
  V per s-tile [128, 4*128]: per-head 128-col block = [V+bv | ones | 0].
  scores^T[k, q] = KTz^T . QT  (k on partitions, q moving, 1024-chunks)
  attn^T = exp(scale * scores^T)  (no max subtraction: |scores| < ~0.3)
  causal: k-tiles beyond the chunk's causal extent skipped, diagonal
  tiles restrict the moving range, one triangular mask multiply on the
  128-wide boundary block.
  out^T[o(+denom), q] accumulates Vaug^T . attn^T in PSUM; ones column
  of Vaug yields the softmax denominator in row 64.
  Epilogue (no PE, no extra PSUM): denom row -> partition-major via DMA
  reshape, reciprocal, back to row-major, GPSIMD partition-broadcast,
  one DVE multiply; output stays [o, q] and the host transposes it
  during the unshard.
"""

import math
import numpy as np
import ml_dtypes

import concourse.bass as bass
import concourse.mybir as mybir
import concourse.tile as tile
from concourse import bacc
from concourse.bass_utils import run_bass_kernel_spmd

B, S, D = 4, 2048, 512
H, A, O = 8, 64, 64
NCORES = 8
HPC = 4                # heads per core
APC = HPC * A          # 256 projection cols per core
VBLK = 128             # per-head V block width (64 out + 1 ones + 63 zero)
SCALE = 1.0 / math.sqrt(S)

F32 = mybir.dt.float32
BF16 = mybir.dt.bfloat16
AF = mybir.ActivationFunctionType
BF_NP = ml_dtypes.bfloat16

QC = 1024              # attention q-chunk width
N_QC = S // QC         # 2
N_DT = D // 128        # 4 contraction tiles
N_SC = S // 512        # 4 s-chunks of 512
N_ST = S // 128        # 16 s-tiles / k-tiles of 128


def build():
    nc = bacc.Bacc("TRN2", target_bir_lowering=False, debug=False,
                   num_devices=NCORES)

    xT_d = nc.dram_tensor("xT", [D, S], BF16, kind="ExternalInput").ap()
    wq_d = nc.dram_tensor("wq", [D, APC], BF16, kind="ExternalInput").ap()
    bq_d = nc.dram_tensor("bq", [2, 128, 1], F32, kind="ExternalInput").ap()
    wk_d = nc.dram_tensor("wk", [D, A], BF16, kind="ExternalInput").ap()
    wv_d = nc.dram_tensor("wv", [D, APC], BF16, kind="ExternalInput").ap()
    bvm_d = nc.dram_tensor("bvm", [128, HPC * VBLK], BF16,
                           kind="ExternalInput").ap()
    out_d = nc.dram_tensor("out", [HPC, N_QC, O, QC], F32,
                       kind="ExternalOutput").ap()

    ngI_d = nc.inline_tensor((np.eye(128) * -1e9).astype(BF_NP), "ngI").ap()
    mlt_np = (np.arange(128)[None, :] < np.arange(128)[:, None])
    mlt_d = nc.inline_tensor(mlt_np.astype(BF_NP), "mlt").ap()

    with tile.TileContext(nc) as tc:
        with tc.tile_pool(name="const", bufs=1) as cpool, \
             tc.tile_pool(name="persist", bufs=1) as ppool, \
             tc.tile_pool(name="attn", bufs=44) as apool, \
             tc.tile_pool(name="fin", bufs=4) as fpool, \
             tc.tile_pool(name="ps_sc", bufs=2, space="PSUM") as ps_sc, \
             tc.tile_pool(name="ps_av", bufs=2, space="PSUM") as ps_av:

            # ---- constants / weights to SBUF ----
            ngI = cpool.tile([128, 128], BF16, tag="ngI", name="ngI")
            mlt = cpool.tile([128, 128], BF16, tag="mlt", name="mlt")
            bvm = cpool.tile([128, HPC * VBLK], BF16, tag="bvm", name="bvm")

            wq_sb, wk_sb, wv_sb = [], [], []
            for dt in range(N_DT):
                wq_sb.append(cpool.tile([128, APC], BF16, tag=f"wq{dt}", name=f"wq{dt}"))
                wk_sb.append(cpool.tile([128, A], BF16, tag=f"wk{dt}", name=f"wk{dt}"))
                wv_sb.append(cpool.tile([128, APC], BF16, tag=f"wv{dt}", name=f"wv{dt}"))
            bq_sb = [cpool.tile([128, 1], F32, tag=f"bq{at}", name=f"bq{at}")
                     for at in range(2)]

            # ---- x^T half tiles (declared here; DMA order below) ----
            xth = [[ppool.tile([128, QC], BF16, tag=f"xt{dt}_{sp}",
                               name=f"xt{dt}_{sp}") for sp in range(2)]
                   for dt in range(N_DT)]

            # DMA order: everything the first projections need comes first,
            # round-robin across the three queues
            SY, SC, GP = nc.sync, nc.scalar, nc.gpsimd
            order = []
            for dt in range(N_DT):        # x^T first half
                order.append((xth[dt][0][:, :],
                              xT_d[dt * 128:(dt + 1) * 128, 0:QC]))
            for dt in range(N_DT):
                order.append((wq_sb[dt][:, :],
                              wq_d[dt * 128:(dt + 1) * 128, :]))
            for at in range(2):
                order.append((bq_sb[at][:, :], bq_d[at]))
            for dt in range(N_DT):
                order.append((wk_sb[dt][:, :],
                              wk_d[dt * 128:(dt + 1) * 128, :]))
            for dt in range(N_DT):        # x^T second half
                order.append((xth[dt][1][:, :],
                              xT_d[dt * 128:(dt + 1) * 128, QC:S]))
            for dt in range(N_DT):
                order.append((wv_sb[dt][:, :],
                              wv_d[dt * 128:(dt + 1) * 128, :]))
            order += [(ngI[:, :], ngI_d[:, :]), (mlt[:, :], mlt_d[:, :]),
                      (bvm[:, :], bvm_d[:, :])]
            for i, (dst, srcap) in enumerate(order):
                [SY, SC, GP][i % 3].dma_start(out=dst, in_=srcap)

            # PE warm-up: full-array dummy matmuls on the first weight tile
            # keep the activity monitor unthrottled while x^T DMAs land
            wu = ps_sc.tile([128, APC], F32, tag="sc", name="wu")
            for i in range(4):
                nc.tensor.matmul(out=wu[:, :], lhsT=wq_sb[0][:, 0:128],
                                 rhs=wq_sb[0][:, :], start=True, stop=True)

            # ---- projections ----
            # QT: [a, s] packed 2 heads per 128-partition tile, half tiles
            qt = [[ppool.tile([128, QC], BF16, tag=f"qt{at}_{sp}",
                              name=f"qt{at}_{sp}") for sp in range(2)]
                  for at in range(2)]
            def qt_proj(at, sp):
                ps = ps_av.tile([128, 1024], F32, tag="av", name="qps")
                for hh in range(2):
                    hs = slice(hh * 512, (hh + 1) * 512)
                    for dt in range(N_DT):
                        nc.tensor.matmul(
                            out=ps[:, hs],
                            lhsT=wq_sb[dt][:, at * 128:(at + 1) * 128],
                            rhs=xth[dt][sp][:, hs],
                            start=(dt == 0), stop=(dt == N_DT - 1))
                nc.vector.tensor_scalar_add(out=qt[at][sp][:, :],
                                            in0=ps[:, :],
                                            scalar1=bq_sb[at][:, :])

            # KT zero-padded into both partition halves: ktz[0] has K^T in
            # rows 0..63 (even heads), ktz[1] in rows 64..127 (odd heads).
            # Full-128 contraction keeps the PE activity monitor warm.
            ktz = [[ppool.tile([128, QC], BF16, tag=f"ktz{i}_{sp}",
                               name=f"ktz{i}_{sp}") for sp in range(2)]
                   for i in range(2)]
            for sp in range(2):
                nc.gpsimd.memset(ktz[0][sp][64:128, :], 0.0)
                nc.gpsimd.memset(ktz[1][sp][0:64, :], 0.0)
            def kt_proj(sp):
                ps = ps_av.tile([64, 1024], F32, tag="av", name="kps")
                for hh in range(2):
                    hs = slice(hh * 512, (hh + 1) * 512)
                    for dt in range(N_DT):
                        nc.tensor.matmul(out=ps[:, hs], lhsT=wk_sb[dt][:, :],
                                         rhs=xth[dt][sp][:, hs],
                                         start=(dt == 0), stop=(dt == N_DT - 1))
                    nc.vector.tensor_copy(ktz[0][sp][0:64, hs], ps[:, hs])
                    nc.vector.tensor_copy(ktz[1][sp][64:128, hs], ps[:, hs])

            # V: per s-tile [128, 4*128]; block = [V+bv | ones | zeros]
            vt = []
            for st in range(N_ST):
                t = ppool.tile([128, HPC * VBLK], BF16, tag=f"v{st}",
                               name=f"v{st}")
                v3 = t[:, :].rearrange("p (h c) -> p h c", h=HPC)
                b3 = bvm[:, :].rearrange("p (h c) -> p h c", h=HPC)
                nc.vector.tensor_copy(v3[:, :, O:VBLK], b3[:, :, O:VBLK])
                vt.append(t)
            def v_proj(st):
                ps = ps_av.tile([128, APC], F32, tag="av", name="vps")
                sp, so = st // 8, (st % 8) * 128
                for dt in range(N_DT):
                    nc.tensor.matmul(
                        out=ps[:, :],
                        lhsT=xth[dt][sp][:, so:so + 128],
                        rhs=wv_sb[dt][:, :],
                        start=(dt == 0), stop=(dt == N_DT - 1))
                for h in range(HPC):
                    nc.vector.tensor_add(
                        out=vt[st][:, h * VBLK:h * VBLK + O],
                        in0=ps[:, h * O:(h + 1) * O],
                        in1=bvm[:, h * VBLK:h * VBLK + O])

            # ---- attention ----
            def sc_exp(h, qc):
                """scores + exp for one (head, q-chunk); returns atn tiles"""
                at = h // 2
                nkj = (QC // 128) * (qc + 1)
                atns = []
                for kj in range(nkj):
                    m = kj - (QC // 128) * qc
                    vs = 128 * m if m > 0 else 0     # valid q start
                    qlo = qc * QC
                    sc_ps = ps_sc.tile([128, QC], F32, tag="sc", name="sc")
                    for hf in range(QC // 512):
                        lo = max(vs, hf * 512)
                        hi = (hf + 1) * 512
                        if lo >= hi:
                            continue
                        nc.tensor.matmul(
                            out=sc_ps[:, lo:hi],
                            lhsT=ktz[h % 2][kj // 8][:, (kj % 8) * 128:
                                                     (kj % 8 + 1) * 128],
                            rhs=qt[at][qc][:, lo:hi],
                            start=True, stop=True)
                    if m >= 0:
                        nc.tensor.matmul(out=sc_ps[:, vs:vs + 128],
                                         lhsT=ngI[:, :], rhs=mlt[:, :],
                                         start=False, stop=True,
                                         skip_group_check=True)
                    atn = apool.tile([128, QC], BF16, tag="atn", name="atn")
                    nc.scalar.activation(out=atn[:, vs:QC],
                                         in_=sc_ps[:, vs:QC],
                                         func=AF.Exp, scale=SCALE)
                    atns.append(atn)
                return atns

            def av_part(h, qc, atns):
                """V-weighted accumulation + epilogue for one chunk"""
                av = ps_av.tile([128, QC], F32, tag="av", name="av")
                nkj = (QC // 128) * (qc + 1)
                for kj in range(nkj):
                    m = kj - (QC // 128) * qc
                    vs = 128 * m if m > 0 else 0
                    for hf in range(QC // 512):
                        lo = max(vs, hf * 512)
                        hi = (hf + 1) * 512
                        if lo >= hi:
                            continue
                        last_kj = nkj - 1 if hf == 1 else \
                            (QC // 128) * qc + 3
                        nc.tensor.matmul(
                            out=av[:, lo:hi],
                            lhsT=vt[kj][:, h * VBLK:(h + 1) * VBLK],
                            rhs=atns[kj][:, lo:hi],
                            start=(kj == 0), stop=(kj == last_kj))

                dr = fpool.tile([1, QC], F32, tag="dr", name="dr")
                nc.vector.tensor_copy(dr[:, :], av[O:O + 1, :])
                drr = fpool.tile([1, QC], F32, tag="drr", name="drr")
                nc.vector.reciprocal_approx_fast(out=drr[:, :], in_=dr[:, :])
                rb = fpool.tile([O, QC], F32, tag="rb", name="rb")
                nc.gpsimd.partition_broadcast(rb[:, :], drr[:, :], channels=O)
                ov = fpool.tile([O, QC], F32, tag="ov", name="ov")
                nc.vector.tensor_mul(ov[:, :], av[0:O, :], rb[:, :])
                nc.scalar.dma_start(out=out_d[h, qc], in_=ov[:, :])

            def attn_chunk(h, qc):
                av_part(h, qc, sc_exp(h, qc))

            # schedule: minimal projection -> early scores/exp, then a
            # chunk-level software pipeline: scores/exp of chunk i+1 are
            # emitted before the AV+epilogue of chunk i so the scalar
            # engine's exp stream never stalls at chunk boundaries
            qt_proj(0, 0)
            kt_proj(0)
            a00 = sc_exp(0, 0)
            a10 = sc_exp(1, 0)
            qt_proj(0, 1)
            qt_proj(1, 0)
            qt_proj(1, 1)
            kt_proj(1)
            a20 = sc_exp(2, 0)
            for st in range(8):
                v_proj(st)
            a01 = sc_exp(0, 1)
            av_part(0, 0, a00)
            for st in range(8, 12):
                v_proj(st)
            a11 = sc_exp(1, 1)
            av_part(1, 0, a10)
            for st in range(12, N_ST):
                v_proj(st)
            a21 = sc_exp(2, 1)
            av_part(2, 0, a20)
            a31 = sc_exp(3, 1)
            av_part(0, 1, a01)
            a30 = sc_exp(3, 0)
            av_part(1, 1, a11)
            av_part(2, 1, a21)
            av_part(3, 1, a31)
            av_part(3, 0, a30)

    nc.compile()
    return nc


_NC = None
LAST_RESULTS = None


def _bvm(bv_slice):
    blk = np.zeros((HPC, VBLK), dtype=np.float32)
    blk[:, :O] = np.asarray(bv_slice, dtype=np.float32).reshape(HPC, O)
    blk[:, O] = 1.0
    return np.ascontiguousarray(np.broadcast_to(
        blk.reshape(1, HPC * VBLK), (128, HPC * VBLK))).astype(BF_NP)


def make_in_maps(x, Wq, bq, Wk, Wv, bv):
    in_maps = []
    for c in range(NCORES):
        b, g = c % 4, c // 4
        cols = slice(g * APC, (g + 1) * APC)
        in_maps.append({
            "xT": np.ascontiguousarray(x[b].T).astype(BF_NP),
            "wq": np.ascontiguousarray(Wq[:, cols]).astype(BF_NP),
            "bq": np.ascontiguousarray(bq[cols].reshape(2, 128, 1)),
            "wk": np.ascontiguousarray(Wk).astype(BF_NP),
            "wv": np.ascontiguousarray(Wv[:, cols]).astype(BF_NP),
            "bvm": _bvm(bv[cols]),
        })
    return in_maps


def gather_out(results):
    out = np.empty((B, S, H * O), dtype=np.float32)
    for c in range(NCORES):
        b, g = c % 4, c // 4
        oc = results[c]["out"]          # [HPC, N_QC, O, QC]
        for h in range(HPC):
            col = g * APC + h * O
            for qc in range(N_QC):
                out[b, qc * QC:(qc + 1) * QC, col:col + O] = oc[h, qc].T
    return out


def kernel(**inputs):
    global _NC, LAST_RESULTS
    x = np.asarray(inputs["x"], dtype=np.float32)
    Wq = np.asarray(inputs["Wq"], dtype=np.float32)
    bq = np.asarray(inputs["bq"], dtype=np.float32)
    Wk = np.asarray(inputs["Wk"], dtype=np.float32)
    Wv = np.asarray(inputs["Wv"], dtype=np.float32)
    bv = np.asarray(inputs["bv"], dtype=np.float32)

    if _NC is None:
        _NC = build()

    in_maps = make_in_maps(x, Wq, bq, Wk, Wv, bv)
    res = run_bass_kernel_spmd(_NC, in_maps, core_ids=list(range(NCORES)))
    LAST_RESULTS = res
    return gather_out(res.results)


# revision 40
# speedup vs baseline: 1.2378x; 1.0048x over previous
"""Multi-head attention (shared key head) on 8 TRN2 NeuronCores.

Sharding: core c handles batch b = c % 4 and head group g = c // 4
(heads 4g..4g+3).  Per-core weights are sliced on host; x is
pre-transposed (and bf16-cast) on host so the device never transposes
the activations.

Device-side per core (bf16 matmul path, fp32 PSUM accumulation):
  xT [512, 2048] -> QT [a, s] (2 tiles, 2 heads each), KT zero-padded
  into two [128, s] variants (low/high partition half) so every scores
  matmul contracts over the full 128 partitions -- half-array matmuls
  keep the PE activity monitor from ever unthrottling the clock.
  V per s-tile [128, 4*128]: per-head 128-col block = [V+bv | ones | 0].
  scores^T[k, q] = KTz^T . QT  (k on partitions, q moving, 1024-chunks)
  attn^T = exp(scale * scores^T)  (no max subtraction: |scores| < ~0.3)
  causal: k-tiles beyond the chunk's causal extent skipped, diagonal
  tiles restrict the moving range, one triangular mask multiply on the
  128-wide boundary block.
  out^T[o(+denom), q] accumulates Vaug^T . attn^T in PSUM; ones column
  of Vaug yields the softmax denominator in row 64.
  Epilogue (no PE, no extra PSUM): denom row -> partition-major via DMA
  reshape, reciprocal, back to row-major, GPSIMD partition-broadcast,
  one DVE multiply; output stays [o, q] and the host transposes it
  during the unshard.
"""

import math
import numpy as np
import ml_dtypes

import concourse.bass as bass
import concourse.mybir as mybir
import concourse.tile as tile
from concourse import bacc
from concourse.bass_utils import run_bass_kernel_spmd

B, S, D = 4, 2048, 512
H, A, O = 8, 64, 64
NCORES = 8
HPC = 4                # heads per core
APC = HPC * A          # 256 projection cols per core
VBLK = 128             # per-head V block width (64 out + 1 ones + 63 zero)
SCALE = 1.0 / math.sqrt(S)

F32 = mybir.dt.float32
BF16 = mybir.dt.bfloat16
AF = mybir.ActivationFunctionType
BF_NP = ml_dtypes.bfloat16

QC = 1024              # attention q-chunk width
N_QC = S // QC         # 2
N_DT = D // 128        # 4 contraction tiles
N_SC = S // 512        # 4 s-chunks of 512
N_ST = S // 128        # 16 s-tiles / k-tiles of 128


def build():
    nc = bacc.Bacc("TRN2", target_bir_lowering=False, debug=False,
                   num_devices=NCORES)

    xT_d = nc.dram_tensor("xT", [D, S], BF16, kind="ExternalInput").ap()
    wq_d = nc.dram_tensor("wq", [D, APC], BF16, kind="ExternalInput").ap()
    bq_d = nc.dram_tensor("bq", [2, 128, 1], F32, kind="ExternalInput").ap()
    wk_d = nc.dram_tensor("wk", [D, A], BF16, kind="ExternalInput").ap()
    wv_d = nc.dram_tensor("wv", [D, APC], BF16, kind="ExternalInput").ap()
    bvm_d = nc.dram_tensor("bvm", [128, HPC * VBLK], BF16,
                           kind="ExternalInput").ap()
    out_d = nc.dram_tensor("out", [HPC, N_QC, O, QC], F32,
                       kind="ExternalOutput").ap()

    ngI_d = nc.inline_tensor((np.eye(128) * -1e9).astype(BF_NP), "ngI").ap()
    mlt_np = (np.arange(128)[None, :] < np.arange(128)[:, None])
    mlt_d = nc.inline_tensor(mlt_np.astype(BF_NP), "mlt").ap()

    with tile.TileContext(nc) as tc:
        with tc.tile_pool(name="const", bufs=1) as cpool, \
             tc.tile_pool(name="persist", bufs=1) as ppool, \
             tc.tile_pool(name="attn", bufs=44) as apool, \
             tc.tile_pool(name="fin", bufs=4) as fpool, \
             tc.tile_pool(name="ps_sc", bufs=2, space="PSUM") as ps_sc, \
             tc.tile_pool(name="ps_av", bufs=2, space="PSUM") as ps_av:

            # ---- constants / weights to SBUF ----
            ngI = cpool.tile([128, 128], BF16, tag="ngI", name="ngI")
            mlt = cpool.tile([128, 128], BF16, tag="mlt", name="mlt")
            bvm = cpool.tile([128, HPC * VBLK], BF16, tag="bvm", name="bvm")

            wq_sb, wk_sb, wv_sb = [], [], []
            for dt in range(N_DT):
                wq_sb.append(cpool.tile([128, APC], BF16, tag=f"wq{dt}", name=f"wq{dt}"))
                wk_sb.append(cpool.tile([128, A], BF16, tag=f"wk{dt}", name=f"wk{dt}"))
                wv_sb.append(cpool.tile([128, APC], BF16, tag=f"wv{dt}", name=f"wv{dt}"))
            bq_sb = [cpool.tile([128, 1], F32, tag=f"bq{at}", name=f"bq{at}")
                     for at in range(2)]

            # ---- x^T half tiles (declared here; DMA order below) ----
            xth = [[ppool.tile([128, QC], BF16, tag=f"xt{dt}_{sp}",
                               name=f"xt{dt}_{sp}") for sp in range(2)]
                   for dt in range(N_DT)]

            # DMA order: everything the first projections need comes first,
            # round-robin across the three queues
            SY, SC, GP = nc.sync, nc.scalar, nc.gpsimd
            order = []
            for dt in range(N_DT):        # x^T first half
                order.append((xth[dt][0][:, :],
                              xT_d[dt * 128:(dt + 1) * 128, 0:QC]))
            for dt in range(N_DT):
                order.append((wq_sb[dt][:, :],
                              wq_d[dt * 128:(dt + 1) * 128, :]))
            for at in range(2):
                order.append((bq_sb[at][:, :], bq_d[at]))
            for dt in range(N_DT):
                order.append((wk_sb[dt][:, :],
                              wk_d[dt * 128:(dt + 1) * 128, :]))
            for dt in range(N_DT):        # x^T second half
                order.append((xth[dt][1][:, :],
                              xT_d[dt * 128:(dt + 1) * 128, QC:S]))
            for dt in range(N_DT):
                order.append((wv_sb[dt][:, :],
                              wv_d[dt * 128:(dt + 1) * 128, :]))
            order += [(ngI[:, :], ngI_d[:, :]), (mlt[:, :], mlt_d[:, :]),
                      (bvm[:, :], bvm_d[:, :])]
            for i, (dst, srcap) in enumerate(order):
                [SY, SC, GP][i % 3].dma_start(out=dst, in_=srcap)

            # PE warm-up: full-array dummy matmuls on the first weight tile
            # keep the activity monitor unthrottled while x^T DMAs land
            wu = ps_sc.tile([128, APC], F32, tag="sc", name="wu")
            for i in range(4):
                nc.tensor.matmul(out=wu[:, :], lhsT=wq_sb[0][:, 0:128],
                                 rhs=wq_sb[0][:, :], start=True, stop=True)

            # ---- projections ----
            # QT: [a, s] packed 2 heads per 128-partition tile, half tiles
            qt = [[ppool.tile([128, QC], BF16, tag=f"qt{at}_{sp}",
                              name=f"qt{at}_{sp}") for sp in range(2)]
                  for at in range(2)]
            def qt_proj(at, sp):
                ps = ps_av.tile([128, 1024], F32, tag="av", name="qps")
                for hh in range(2):
                    hs = slice(hh * 512, (hh + 1) * 512)
                    for dt in range(N_DT):
                        nc.tensor.matmul(
                            out=ps[:, hs],
                            lhsT=wq_sb[dt][:, at * 128:(at + 1) * 128],
                            rhs=xth[dt][sp][:, hs],
                            start=(dt == 0), stop=(dt == N_DT - 1))
                nc.vector.tensor_scalar_add(out=qt[at][sp][:, :],
                                            in0=ps[:, :],
                                            scalar1=bq_sb[at][:, :])

            # KT zero-padded into both partition halves: ktz[0] has K^T in
            # rows 0..63 (even heads), ktz[1] in rows 64..127 (odd heads).
            # Full-128 contraction keeps the PE activity monitor warm.
            ktz = [[ppool.tile([128, QC], BF16, tag=f"ktz{i}_{sp}",
                               name=f"ktz{i}_{sp}") for sp in range(2)]
                   for i in range(2)]
            for sp in range(2):
                nc.gpsimd.memset(ktz[0][sp][64:128, :], 0.0)
                nc.gpsimd.memset(ktz[1][sp][0:64, :], 0.0)
            def kt_proj(sp):
                ps = ps_av.tile([64, 1024], F32, tag="av", name="kps")
                for hh in range(2):
                    hs = slice(hh * 512, (hh + 1) * 512)
                    for dt in range(N_DT):
                        nc.tensor.matmul(out=ps[:, hs], lhsT=wk_sb[dt][:, :],
                                         rhs=xth[dt][sp][:, hs],
                                         start=(dt == 0), stop=(dt == N_DT - 1))
                    nc.vector.tensor_copy(ktz[0][sp][0:64, hs], ps[:, hs])
                    nc.vector.tensor_copy(ktz[1][sp][64:128, hs], ps[:, hs])

            # V: per s-tile [128, 4*128]; block = [V+bv | ones | zeros]
            vt = []
            for st in range(N_ST):
                t = ppool.tile([128, HPC * VBLK], BF16, tag=f"v{st}",
                               name=f"v{st}")
                v3 = t[:, :].rearrange("p (h c) -> p h c", h=HPC)
                b3 = bvm[:, :].rearrange("p (h c) -> p h c", h=HPC)
                nc.vector.tensor_copy(v3[:, :, O:VBLK], b3[:, :, O:VBLK])
                vt.append(t)
            def v_proj(st):
                ps = ps_av.tile([128, APC], F32, tag="av", name="vps")
                sp, so = st // 8, (st % 8) * 128
                for dt in range(N_DT):
                    nc.tensor.matmul(
                        out=ps[:, :],
                        lhsT=xth[dt][sp][:, so:so + 128],
                        rhs=wv_sb[dt][:, :],
                        start=(dt == 0), stop=(dt == N_DT - 1))
                for h in range(HPC):
                    nc.vector.tensor_add(
                        out=vt[st][:, h * VBLK:h * VBLK + O],
                        in0=ps[:, h * O:(h + 1) * O],
                        in1=bvm[:, h * VBLK:h * VBLK + O])

            # ---- attention ----
            def sc_exp(h, qc):
                """scores + exp for one (head, q-chunk); returns atn tiles"""
                at = h // 2
                nkj = (QC // 128) * (qc + 1)
                atns = []
                for kj in range(nkj):
                    m = kj - (QC // 128) * qc
                    vs = 128 * m if m > 0 else 0     # valid q start
                    qlo = qc * QC
                    sc_ps = ps_sc.tile([128, QC], F32, tag="sc", name="sc")
                    for hf in range(QC // 512):
                        lo = max(vs, hf * 512)
                        hi = (hf + 1) * 512
                        if lo >= hi:
                            continue
                        nc.tensor.matmul(
                            out=sc_ps[:, lo:hi],
                            lhsT=ktz[h % 2][kj // 8][:, (kj % 8) * 128:
                                                     (kj % 8 + 1) * 128],
                            rhs=qt[at][qc][:, lo:hi],
                            start=True, stop=True)
                    if m >= 0:
                        nc.tensor.matmul(out=sc_ps[:, vs:vs + 128],
                                         lhsT=ngI[:, :], rhs=mlt[:, :],
                                         start=False, stop=True,
                                         skip_group_check=True)
                    atn = apool.tile([128, QC], BF16, tag="atn", name="atn")
                    nc.scalar.activation(out=atn[:, vs:QC],
                                         in_=sc_ps[:, vs:QC],
                                         func=AF.Exp, scale=SCALE)
                    atns.append(atn)
                return atns

            def av_part(h, qc, atns):
                """V-weighted accumulation + epilogue for one chunk"""
                av = ps_av.tile([128, QC], F32, tag="av", name="av")
                nkj = (QC // 128) * (qc + 1)
                for kj in range(nkj):
                    m = kj - (QC // 128) * qc
                    vs = 128 * m if m > 0 else 0
                    for hf in range(QC // 512):
                        lo = max(vs, hf * 512)
                        hi = (hf + 1) * 512
                        if lo >= hi:
                            continue
                        last_kj = nkj - 1 if hf == 1 else \
                            (QC // 128) * qc + 3
                        nc.tensor.matmul(
                            out=av[:, lo:hi],
                            lhsT=vt[kj][:, h * VBLK:(h + 1) * VBLK],
                            rhs=atns[kj][:, lo:hi],
                            start=(kj == 0), stop=(kj == last_kj))

                dr = fpool.tile([1, QC], F32, tag="dr", name="dr")
                nc.vector.tensor_copy(dr[:, :], av[O:O + 1, :])
                drr = fpool.tile([1, QC], F32, tag="drr", name="drr")
                nc.vector.reciprocal_approx_fast(out=drr[:, :], in_=dr[:, :])
                rb = fpool.tile([O, QC], F32, tag="rb", name="rb")
                nc.gpsimd.partition_broadcast(rb[:, :], drr[:, :], channels=O)
                ov = fpool.tile([O, QC], F32, tag="ov", name="ov")
                nc.vector.tensor_mul(ov[:, :], av[0:O, :], rb[:, :])
                nc.scalar.dma_start(out=out_d[h, qc], in_=ov[:, :])

            def attn_chunk(h, qc):
                av_part(h, qc, sc_exp(h, qc))

            # schedule: minimal projection -> early scores/exp, then a
            # chunk-level software pipeline: scores/exp of chunk i+1 are
            # emitted before the AV+epilogue of chunk i so the scalar
            # engine's exp stream never stalls at chunk boundaries
            qt_proj(0, 0)
            kt_proj(0)
            a00 = sc_exp(0, 0)
            a10 = sc_exp(1, 0)
            qt_proj(0, 1)
            qt_proj(1, 0)
            qt_proj(1, 1)
            kt_proj(1)
            a20 = sc_exp(2, 0)
            for st in range(8):
                v_proj(st)
            a01 = sc_exp(0, 1)
            av_part(0, 0, a00)
            for st in range(8, 12):
                v_proj(st)
            a11 = sc_exp(1, 1)
            av_part(1, 0, a10)
            for st in range(12, N_ST):
                v_proj(st)
            a21 = sc_exp(2, 1)
            av_part(2, 0, a20)
            av_part(0, 1, a01)
            a31 = sc_exp(3, 1)
            av_part(1, 1, a11)
            a30 = sc_exp(3, 0)
            av_part(2, 1, a21)
            av_part(3, 1, a31)
            av_part(3, 0, a30)

    nc.compile()
    return nc


_NC = None
LAST_RESULTS = None


def _bvm(bv_slice):
    blk = np.zeros((HPC, VBLK), dtype=np.float32)
    blk[:, :O] = np.asarray(bv_slice, dtype=np.float32).reshape(HPC, O)
    blk[:, O] = 1.0
    return np.ascontiguousarray(np.broadcast_to(
        blk.reshape(1, HPC * VBLK), (128, HPC * VBLK))).astype(BF_NP)


def make_in_maps(x, Wq, bq, Wk, Wv, bv):
    in_maps = []
    for c in range(NCORES):
        b, g = c % 4, c // 4
        cols = slice(g * APC, (g + 1) * APC)
        in_maps.append({
            "xT": np.ascontiguousarray(x[b].T).astype(BF_NP),
            "wq": np.ascontiguousarray(Wq[:, cols]).astype(BF_NP),
            "bq": np.ascontiguousarray(bq[cols].reshape(2, 128, 1)),
            "wk": np.ascontiguousarray(Wk).astype(BF_NP),
            "wv": np.ascontiguousarray(Wv[:, cols]).astype(BF_NP),
            "bvm": _bvm(bv[cols]),
        })
    return in_maps


def gather_out(results):
    out = np.empty((B, S, H * O), dtype=np.float32)
    for c in range(NCORES):
        b, g = c % 4, c // 4
        oc = results[c]["out"]          # [HPC, N_QC, O, QC]
        for h in range(HPC):
            col = g * APC + h * O
            for qc in range(N_QC):
                out[b, qc * QC:(qc + 1) * QC, col:col + O] = oc[h, qc].T
    return out


def kernel(**inputs):
    global _NC, LAST_RESULTS
    x = np.asarray(inputs["x"], dtype=np.float32)
    Wq = np.asarray(inputs["Wq"], dtype=np.float32)
    bq = np.asarray(inputs["bq"], dtype=np.float32)
    Wk = np.asarray(inputs["Wk"], dtype=np.float32)
    Wv = np.asarray(inputs["Wv"], dtype=np.float32)
    bv = np.asarray(inputs["bv"], dtype=np.float32)

    if _NC is None:
        _NC = build()

    in_maps = make_in_maps(x, Wq, bq, Wk, Wv, bv)
    res = run_bass_kernel_spmd(_NC, in_maps, core_ids=list(range(NCORES)))
    LAST_RESULTS = res
    return gather_out(res.results)
